# revision 28
# baseline (speedup 1.0000x reference)
"""TRN2 Bass kernel for nn_Attention (RMSNorm + QKV + softmax attention + out-proj).

Sharding: 8 cores = 2 batches x 4 head-pairs. Core c handles batch c//4 and
heads (2*(c%4), 2*(c%4)+1). Each core computes its partial out-projection
(contracting only its 128 rows of dim_inner); host sums the 4 partials per batch.

Per-core pipeline (all matmuls f32r, 1 cycle/row at free>=256):
  A) stream tokens [128,512] blocks: RMSNorm stats on DVE, rstd scale, PE
     transpose -> x^T [512, 4096] (rotating [128,4,512] per 512-token chunk)
  B) QKV^T = w^T @ x^T (PE), V^T transposed back to V-natural with a ones
     column appended per head (gives softmax denominator for free)
  C) flash-style attention per (head, query-block of 512):
     S^T tiles [128j, 512i] on PE -> exp on ACT (no max subtraction; scores
     bounded ~45, exp fits fp32) -> o^T accum [65, 512] on PE (row 64 = l)
     -> linv = 1/l (DVE) -> broadcast via ones matmul -> O^T = o^T * linv_b
  D) out-proj: out[i,:] += O^T_h.T @ w_out_h per head, DMA partial out.

Engine discipline: walrus here allows only ONE semaphore wait per instruction,
so producers are assigned to engines such that every instruction has at most
one un-observed cross-engine dependency (see joins / scratch-copy tricks).
"""
import sys
sys.path.insert(0, "/opt/trn_rl_repo")
import numpy as np

B, N, D = 2, 4096, 512
H, DH = 8, 64
DI = H * DH
NCORES = 8
EPS = 1.1920929e-07  # float32 eps (torch nn.RMSNorm default)

_prog_cache = {}


def _patch_drain(tile_mod, mybir):
    """Split the multi-wait tail drain into a chain of single-wait drains
    (this walrus build rejects >1 sync wait per instruction)."""
    if getattr(tile_mod.TileContext, "_drain_patched", False):
        return

    def _patched(self, tick_clock, wait_clock):
        from concourse.vector_clock import ScopedClock
        nc = self.nc
        drain_inst = nc.sync.drain()
        wait_clock.add_sem_waits(drain_inst.ins, ScopedClock({None: tick_clock.global_clock}))
        si = drain_inst.ins.sync_info
        if si is not None and si.on_wait and len(si.on_wait) > 1:
            waits = list(si.on_wait)
            drain_inst.ins.sync_info = mybir.SyncInfo(
                on_wait=waits[:1], on_update=list(si.on_update or []))
            for w in waits[1:]:
                d2 = nc.sync.drain()
                d2.ins.sync_info = mybir.SyncInfo(on_wait=[w], on_update=[])
        nc.all_engine_barrier()
        assert self.sems is not None
        popped = nc._tile_sem_poison_stack.pop()
        assert popped is self._sem_poison
        nc.clear_and_free_semaphores(list(self.sems.allocated().values()))
        nc.all_engine_barrier()

    tile_mod.TileContext._drain_and_barrier = _patched
    tile_mod.TileContext._drain_patched = True


def build_program():
    import concourse.bass as bass
    import concourse.tile as tile
    from concourse import mybir
    from concourse.masks import make_identity

    _patch_drain(tile, mybir)

    F32 = mybir.dt.float32
    F32R = mybir.dt.float32r
    BF16 = mybir.dt.bfloat16
    AF = mybir.ActivationFunctionType
    ALU = mybir.AluOpType
    AX = mybir.AxisListType

    NB = N // 128           # 32 token blocks of 128
    NIC = N // 512          # 8 chunks of 512 tokens
    NJT = N // 128          # 32 key tiles of 128

    nc = bass.Bass(trn_type="TRN2", target_bir_lowering=False)

    tok = nc.dram_tensor("tok", [N, D], F32, kind="ExternalInput")
    wq = nc.dram_tensor("wq", [128, 4, 128], F32R, kind="ExternalInput")
    wk = nc.dram_tensor("wk", [128, 4, 128], F32R, kind="ExternalInput")
    wv = nc.dram_tensor("wv", [128, 4, 128], F32R, kind="ExternalInput")
    wo0 = nc.dram_tensor("wo0", [64, 512], F32R, kind="ExternalInput")
    wo1 = nc.dram_tensor("wo1", [64, 512], F32R, kind="ExternalInput")
    out_part = nc.dram_tensor("out_part", [N, D], F32, kind="ExternalOutput")

    tok_r = tok.rearrange("(ic t p) d -> ic p t d", t=4, p=128)
    out_r = out_part.rearrange("(ib t p) e -> ib p t e", t=4, p=128)

    with tile.TileContext(nc) as tc:
        with tc.tile_pool(name="consts", bufs=1) as consts, \
             tc.tile_pool(name="big", bufs=1) as big, \
             tc.tile_pool(name="wpool", bufs=1) as wpool:

            # ---- constants ----
            ident_f = consts.tile([128, 128], F32)
            make_identity(nc, ident_f)
            ident = consts.tile([128, 128], F32R)
            nc.vector.tensor_copy(ident, ident_f)
            eps_t = consts.tile([128, 1], F32)
            nc.vector.memset(eps_t, EPS)
            ones_f = consts.tile([128, 64], F32)
            nc.vector.memset(ones_f, 1.0)
            ones_r = consts.tile([1, 64], F32R)
            nc.vector.tensor_copy(ones_r, ones_f[0:1, :])

            # ---- weights ----
            wq_sb = wpool.tile([128, 4, 128], F32R)
            wk_sb = wpool.tile([128, 4, 128], F32R)
            wv_sb = wpool.tile([128, 4, 128], F32R)
            wo0_sb = wpool.tile([64, 512], F32R)
            wo1_sb = wpool.tile([64, 512], F32R)
            nc.sync.dma_start(out=wq_sb, in_=wq[:, :, :])
            nc.sync.dma_start(out=wk_sb, in_=wk[:, :, :])
            nc.sync.dma_start(out=wv_sb, in_=wv[:, :, :])
            nc.sync.dma_start(out=wo0_sb, in_=wo0[:, :])
            nc.sync.dma_start(out=wo1_sb, in_=wo1[:, :])

            # ---- persistent big buffers ----
            QT = big.tile([128, N], F32R)       # [2 heads x 64 qdims, n]
            KT = big.tile([128, N], F32R)
            Vb = big.tile([128, NJT, 2, 65], F32R)  # per j-tile: [v(64)|ones] per head
            nc.vector.tensor_copy(
                Vb[:, :, :, 64:65],
                ones_f.rearrange("p (a b c) -> p a b c", a=NJT, b=2, c=1))

            GROUPS = []
            jt0 = 0
            while jt0 < NJT:
                g = min(3, NJT - jt0)
                GROUPS.append(list(range(jt0, jt0 + g)))
                jt0 += g
            from contextlib import ExitStack
            outer_ctx = ExitStack()
            ptp = outer_ctx.enter_context(tc.tile_pool(name="pt_pool", bufs=3))
            ops = outer_ctx.enter_context(tc.tile_pool(name="o_psum", bufs=1, space="PSUM"))
            mixps = outer_ctx.enter_context(tc.tile_pool(name="mix_psum", bufs=1, space="PSUM"))
            first_pv = [True]

            def emit_flash_group(o_ps, hl, ib, grp, st):
                h0 = hl * 64
                g = len(grp)
                for k, jt in enumerate(grp):
                    nc.tensor.matmul(
                        st[:, k, :],
                        KT[h0:h0 + 64, jt * 128:(jt + 1) * 128],
                        QT[h0:h0 + 64, ib * 512:(ib + 1) * 512],
                        start=True, stop=True)
                pt = ptp.tile([128, 3, 512], F32R, tag="pt", name="ptg")
                nc.scalar.activation(
                    pt[:, 0:g, :].rearrange("p a b -> p (a b)"),
                    st[:, 0:g, :].rearrange("p a b -> p (a b)"),
                    AF.Exp)
                for k, jt in enumerate(grp):
                    nc.tensor.matmul(
                        o_ps, Vb[:, jt, hl, :], pt[:, k, :],
                        start=first_pv[0], stop=(jt == NJT - 1))
                    first_pv[0] = False

            with tc.tile_pool(name="ab_sbuf", bufs=3) as abp, \
                 tc.tile_pool(name="ab_stats", bufs=4) as stp, \
                 tc.tile_pool(name="ab_psum", bufs=3, space="PSUM") as abps, \
                 tc.tile_pool(name="qk_psum", bufs=2, space="PSUM") as qkps, \
                 tc.tile_pool(name="scr_psum", bufs=1, space="PSUM") as scrps:

                # PE joins: absorb each weight-DMA semaphore with a tiny bf16 matmul
                scr = scrps.tile([2, 2], F32, tag="scr", name="scrj")
                for i, wtile in enumerate((wq_sb, wk_sb, wv_sb, wo0_sb, wo1_sb)):
                    # high bf16 halves of 2 consecutive f32 weights (low halves can be NaN bits)
                    if len(wtile.shape) == 3:
                        src = wtile[0:1, 0:1, 0:2].bitcast(BF16)[:, 0, 1::2]
                    else:
                        src = wtile[0:1, 0:2].bitcast(BF16)[:, 1::2]
                    nc.tensor.matmul(scr, src, src, start=(i == 0), stop=(i == 4))

                for ic in range(NIC):
                    tok4 = abp.tile([128, 4, 512], F32, tag="tok4")
                    nc.gpsimd.dma_start(out=tok4, in_=tok_r[ic])
                    xt = abp.tile([128, 4, 512], F32R, tag="xt")
                    for t in range(4):
                        stats = stp.tile([128, 6], F32, tag="stats")
                        mv = stp.tile([128, 2], F32, tag="mv")
                        ms = stp.tile([128, 1], F32, tag="ms")
                        s_t = stp.tile([128, 1], F32, tag="s_t")
                        rstd = stp.tile([128, 1], F32, tag="rstd")
                        nc.vector.bn_stats(stats, tok4[:, t, :])
                        nc.vector.bn_aggr(mv, stats)
                        # E[x^2] = mean^2 + var
                        nc.vector.scalar_tensor_tensor(
                            ms, mv[:, 0:1], mv[:, 0:1], mv[:, 1:2],
                            op0=ALU.mult, op1=ALU.add)
                        nc.scalar.activation(s_t, ms, AF.Sqrt, bias=eps_t, scale=1.0)
                        nc.vector.reciprocal(rstd, s_t)
                        # alternate the scale between DVE and GpSimd to
                        # balance the two phase-A bottleneck engines; per-t
                        # tiles so each transpose depends only on its own scale
                        xn = stp.tile([128, 512], F32R, tag="xn")
                        eng = nc.vector if t % 2 == 0 else nc.gpsimd
                        eng.tensor_scalar_mul(xn, in0=tok4[:, t, :], scalar1=rstd)
                        tp = abps.tile([128, 4, 128], F32R, tag="tp")
                        for c in range(4):
                            nc.tensor.transpose(tp[:, c, :], xn[:, c * 128:(c + 1) * 128], ident)
                        nc.scalar.copy(xt[:, :, t * 128:(t + 1) * 128], tp)

                    # QKV^T for this 512-token chunk (V first: see DVE ordering note)
                    vt = abp.tile([128, 512], F32R, tag="vt")
                    for wtile, dst in ((wv_sb, None), (wq_sb, QT), (wk_sb, KT)):
                        ps = qkps.tile([128, 512], F32, tag="qk")
                        for c in range(4):
                            nc.tensor.matmul(ps, wtile[:, c, :], xt[:, c, :],
                                             start=(c == 0), stop=(c == 3))
                        if dst is None:
                            nc.vector.tensor_copy(vt, ps)
                        else:
                            nc.vector.tensor_copy(dst[:, ic * 512:(ic + 1) * 512], ps)
                    # V^T -> V natural into Vb (j on partitions), all 4 j-tiles
                    vtp = abps.tile([128, 4, 128], F32R, tag="tp")
                    for jl in range(4):
                        nc.tensor.transpose(vtp[:, jl, :], vt[:, jl * 128:(jl + 1) * 128], ident)
                    nc.vector.tensor_copy(
                        Vb[:, ic * 4:(ic + 1) * 4, :, 0:64],
                        vtp.rearrange("p jl (h v) -> p jl h v", h=2))

            # ---- phase C + D ----
            with tc.tile_pool(name="c_sbuf", bufs=2) as cp, \
                 tc.tile_pool(name="osb_pool", bufs=2) as osbp, \
                 tc.tile_pool(name="lc_pool", bufs=2) as lcp, \
                 tc.tile_pool(name="st_psum", bufs=2, space="PSUM") as stps:

                saved = {}

                def emit_tail(o_ps, ib, hl):
                    # l = row 64 of o_ps; o^T = rows 0..63. 1/l is folded into
                    # the out-projection extraction (per-partition scalar),
                    # so we need l transposed to [i-partitions, 1] — done with
                    # a small SBUF->SBUF DMA scatter.
                    l_sb = cp.tile([1, 512], F32, tag="l_sb")
                    nc.vector.tensor_copy(l_sb, o_ps[64:65, :])
                    o_sb = osbp.tile([64, 512], F32R, tag=f"osb{hl}")
                    nc.vector.tensor_copy(o_sb, o_ps[0:64, :])
                    linv_pre = cp.tile([128, 4], F32, tag="linv_pre")
                    for it in range(4):
                        nc.sync.dma_start(
                            out=linv_pre[:, it:it + 1],
                            in_=l_sb[0:1, it * 128:(it + 1) * 128])
                    linv_col = lcp.tile([128, 4], F32, tag=f"lc{hl}")
                    nc.vector.reciprocal(linv_col, linv_pre)
                    saved[(ib, hl)] = (o_sb, linv_col)

                def emit_outproj_tile(ib, it, out_sb, tmp_on_act=False,
                                      use_st=False):
                    o_sb0, lc0 = saved[(ib, 0)]
                    o_sb1, lc1 = saved[(ib, 1)]
                    pool = stps if use_st else mixps
                    tg = "st" if use_st else "mix"
                    op_ps = pool.tile([128, 512], F32, tag=tg)
                    nc.tensor.matmul(op_ps, o_sb0[:, it * 128:(it + 1) * 128],
                                     wo0_sb, start=True, stop=True)
                    tmp = cp.tile([128, 512], F32, tag="tmp")
                    if tmp_on_act:
                        nc.scalar.mul(tmp, op_ps, lc0[:, it:it + 1])
                    else:
                        nc.vector.tensor_scalar_mul(tmp, in0=op_ps, scalar1=lc0[:, it:it + 1])
                    op_ps2 = pool.tile([128, 512], F32, tag=tg)
                    nc.tensor.matmul(op_ps2, o_sb1[:, it * 128:(it + 1) * 128],
                                     wo1_sb, start=True, stop=True)
                    nc.vector.scalar_tensor_tensor(
                        out_sb[:, it, :], op_ps2, lc1[:, it:it + 1], tmp,
                        op0=ALU.mult, op1=ALU.add)

                prev = None
                pending_op = None  # i-block whose out-projection is owed
                out_sbs = {}
                for ib in range(NIC):
                    for hl in range(2):
                        o_ps = ops.tile([65, 512], F32, tag="o")
                        first_pv[0] = True
                        g_start = 0
                        for g_idx in range(g_start, len(GROUPS)):
                            grp = GROUPS[g_idx]
                            st = stps.tile([128, 3, 512], F32, tag="st")
                            emit_flash_group(o_ps, hl, ib, grp, st)
                            # software-pipelined: previous iteration's tail
                            # after group 0, one out-proj i-tile per group
                            if g_idx == g_start and prev is not None:
                                emit_tail(*prev)
                                if prev[2] == 1:
                                    pending_op = prev[1]
                                    out_sbs[pending_op] = cp.tile(
                                        [128, 4, 512], F32, tag="out_sb",
                                        name=f"outsb{pending_op}")
                            elif (g_start + 1 <= g_idx <= g_start + 4
                                  and pending_op is not None):
                                emit_outproj_tile(pending_op, g_idx - g_start - 1,
                                                  out_sbs[pending_op])
                                if g_idx == g_start + 4:
                                    done = pending_op
                                    nc.sync.dma_start(out=out_r[done],
                                                      in_=out_sbs.pop(done))
                                    saved.pop((done, 0))
                                    saved.pop((done, 1))
                                    pending_op = None
                        prev = (o_ps, ib, hl)

                # final tail + out-projection (use the now-free st slots so the
                # 8 matmuls don't serialize on one PSUM bank)
                emit_tail(*prev)
                fin = prev[1]
                out_fin = cp.tile([128, 4, 512], F32, tag="out_sb")
                for it in range(4):
                    emit_outproj_tile(fin, it, out_fin, tmp_on_act=True,
                                      use_st=True)
                nc.sync.dma_start(out=out_r[fin], in_=out_fin)
            outer_ctx.close()

    fix_waits_nc(nc, mybir)
    return nc


def fix_waits_nc(nc, mybir):
    """Post-pass over the scheduled program: (1) remove semaphore waits that
    are transitively implied by earlier waits (Tile emits per-proc-minimal,
    not transitively-minimal, waits), (2) split any instruction still
    carrying more than one wait by injecting single-wait NoOps in front of
    it — this walrus build rejects >1 sync wait per instruction.
    Mutates nc in place so CoreSim and hardware run identical sync."""
    nop_id = [0]

    def _is_ge(w):
        return w.sync_type == "semaphore" and w.wait_mode == "sem-ge-imm"

    for fn in nc.m.functions:
        for blk in fn.blocks:
            insts = list(blk.instructions)
            n = len(insts)

            producers = {}
            cum = {}
            nonmono = set()  # sems ever decremented: counter logic invalid
            for idx, inst in enumerate(insts):
                si = inst.sync_info
                for u in (si.on_update if si else []) or []:
                    if u.sync_type != "semaphore":
                        continue
                    sid = u.id
                    if u.update_mode != "sem-inc":
                        nonmono.add(sid)
                        continue
                    cum[sid] = cum.get(sid, 0) + int(u.update_value)
                    producers.setdefault(sid, []).append((cum[sid], idx))

            def producer_of(sid, val):
                for cv, idx in producers.get(sid, ()):
                    if cv >= val:
                        return idx
                return None

            prev_eng = [None] * n
            last = {}
            for idx, inst in enumerate(insts):
                e = inst.engine
                prev_eng[idx] = last.get(e)
                last[e] = idx

            def get_waits(inst):
                si = inst.sync_info
                return list(si.on_wait) if si and si.on_wait else []

            def is_ge(w):
                return _is_ge(w) and w.id not in nonmono

            know = [dict() for _ in range(n)]
            for _ in range(3):
                changed = False
                for idx, inst in enumerate(insts):
                    k = dict(know[prev_eng[idx]]) if prev_eng[idx] is not None else {}
                    for w in get_waits(inst):
                        if not is_ge(w):
                            continue
                        sid, val = w.id, int(w.wait_value)
                        if k.get(sid, -1) < val:
                            k[sid] = val
                        p = producer_of(sid, val)
                        if p is not None:
                            for s2, v2 in know[p].items():
                                if k.get(s2, -1) < v2:
                                    k[s2] = v2
                    if k != know[idx]:
                        know[idx] = k
                        changed = True
                if not changed:
                    break

            new_insts = []
            dirty = False
            for idx, inst in enumerate(insts):
                si = inst.sync_info
                waits = get_waits(inst)
                if si is not None and waits:
                    base = dict(know[prev_eng[idx]]) if prev_eng[idx] is not None else {}
                    kept = []
                    for w in waits:
                        if is_ge(w):
                            sid, val = w.id, int(w.wait_value)
                            if base.get(sid, -1) >= val:
                                continue
                            base[sid] = val
                            p = producer_of(sid, val)
                            if p is not None:
                                for s2, v2 in know[p].items():
                                    if base.get(s2, -1) < v2:
                                        base[s2] = v2
                        kept.append(w)
                    if len(kept) != len(waits) or len(kept) > 1:
                        dirty = True
                        for w in kept[:-1]:
                            nop_id[0] += 1
                            nop = mybir.InstNoOp(
                                name=f"I-waitfix-{nop_id[0]}", ins=[], outs=[])
                            nop.engine = inst.engine
                            nop.sync_info = mybir.SyncInfo(on_wait=[w], on_update=[])
                            nc.register_instruction(nop)
                            new_insts.append(nop)
                        inst.sync_info = mybir.SyncInfo(
                            on_wait=kept[-1:],
                            on_update=list(si.on_update or []))
                new_insts.append(inst)
            if dirty:
                blk.instructions = new_insts


def get_program():
    if "nc" not in _prog_cache:
        _prog_cache["nc"] = build_program()
    return _prog_cache["nc"]


def _prep_inputs(tokens, norm_weight, w_qkv, w_out):
    tokens = np.ascontiguousarray(np.asarray(tokens, dtype=np.float32))
    norm_weight = np.asarray(norm_weight, dtype=np.float32)
    w_qkv = np.asarray(w_qkv, dtype=np.float32)
    w_out = np.asarray(w_out, dtype=np.float32)

    wp = w_qkv * norm_weight[:, None]  # fold RMSNorm weight into qkv weights

    in_maps = []
    for c in range(NCORES):
        b = c // 4
        h0 = 2 * (c % 4)
        cols = np.r_[h0 * DH:(h0 + 1) * DH, (h0 + 1) * DH:(h0 + 2) * DH]
        m = {}
        m["tok"] = tokens[b]
        for name, off in (("wq", 0), ("wk", DI), ("wv", 2 * DI)):
            w = wp[:, off + h0 * DH: off + (h0 + 2) * DH]       # [512, 128]
            m[name] = np.ascontiguousarray(
                w.reshape(4, 128, 128).transpose(1, 0, 2))       # [128, 4, 128]
        rows = w_out[h0 * DH:(h0 + 2) * DH, :]                   # [128, 512]
        m["wo0"] = np.ascontiguousarray(rows[0:64])
        m["wo1"] = np.ascontiguousarray(rows[64:128])
        in_maps.append(m)
    return in_maps


def run(tokens, norm_weight, w_qkv, w_out, trace=False):
    from concourse.bass_utils import run_bass_kernel_spmd
    nc = get_program()
    in_maps = _prep_inputs(tokens, norm_weight, w_qkv, w_out)
    res = run_bass_kernel_spmd(nc, in_maps, core_ids=list(range(NCORES)), trace=trace)
    parts = [res.results[c]["out_part"] for c in range(NCORES)]
    out = np.empty((B, N, D), dtype=np.float32)
    for b in range(B):
        out[b] = parts[4 * b] + parts[4 * b + 1] + parts[4 * b + 2] + parts[4 * b + 3]
    return out, res


def kernel(tokens, norm_weight, w_qkv, w_out):
    out, _ = run(tokens, norm_weight, w_qkv, w_out, trace=False)
    return out


# revision 31
# speedup vs baseline: 1.0195x; 1.0195x over previous
"""TRN2 Bass kernel for nn_Attention (RMSNorm + QKV + softmax attention + out-proj).

Sharding: 8 cores = 2 batches x 4 head-pairs. Core c handles batch c//4 and
heads (2*(c%4), 2*(c%4)+1). Each core computes its partial out-projection
(contracting only its 128 rows of dim_inner); host sums the 4 partials per batch.

Per-core pipeline (all matmuls f32r, 1 cycle/row at free>=256):
  A) stream tokens [128,512] blocks: RMSNorm stats on DVE, rstd scale, PE
     transpose -> x^T [512, 4096] (rotating [128,4,512] per 512-token chunk)
  B) QKV^T = w^T @ x^T (PE), V^T transposed back to V-natural with a ones
     column appended per head (gives softmax denominator for free)
  C) flash-style attention per (head, query-block of 512):
     S^T tiles [128j, 512i] on PE -> exp on ACT (no max subtraction; scores
     bounded ~45, exp fits fp32) -> o^T accum [65, 512] on PE (row 64 = l)
     -> linv = 1/l (DVE) -> broadcast via ones matmul -> O^T = o^T * linv_b
  D) out-proj: out[i,:] += O^T_h.T @ w_out_h per head, DMA partial out.

Engine discipline: walrus here allows only ONE semaphore wait per instruction,
so producers are assigned to engines such that every instruction has at most
one un-observed cross-engine dependency (see joins / scratch-copy tricks).
"""
import sys
sys.path.insert(0, "/opt/trn_rl_repo")
import numpy as np

B, N, D = 2, 4096, 512
H, DH = 8, 64
DI = H * DH
NCORES = 8
EPS = 1.1920929e-07  # float32 eps (torch nn.RMSNorm default)

_prog_cache = {}


def _patch_drain(tile_mod, mybir):
    """Split the multi-wait tail drain into a chain of single-wait drains
    (this walrus build rejects >1 sync wait per instruction)."""
    if getattr(tile_mod.TileContext, "_drain_patched", False):
        return

    def _patched(self, tick_clock, wait_clock):
        from concourse.vector_clock import ScopedClock
        nc = self.nc
        drain_inst = nc.sync.drain()
        wait_clock.add_sem_waits(drain_inst.ins, ScopedClock({None: tick_clock.global_clock}))
        si = drain_inst.ins.sync_info
        if si is not None and si.on_wait and len(si.on_wait) > 1:
            waits = list(si.on_wait)
            drain_inst.ins.sync_info = mybir.SyncInfo(
                on_wait=waits[:1], on_update=list(si.on_update or []))
            for w in waits[1:]:
                d2 = nc.sync.drain()
                d2.ins.sync_info = mybir.SyncInfo(on_wait=[w], on_update=[])
        nc.all_engine_barrier()
        assert self.sems is not None
        popped = nc._tile_sem_poison_stack.pop()
        assert popped is self._sem_poison
        nc.clear_and_free_semaphores(list(self.sems.allocated().values()))
        nc.all_engine_barrier()

    tile_mod.TileContext._drain_and_barrier = _patched
    tile_mod.TileContext._drain_patched = True


def build_program():
    import concourse.bass as bass
    import concourse.tile as tile
    from concourse import mybir
    from concourse.masks import make_identity

    _patch_drain(tile, mybir)

    F32 = mybir.dt.float32
    F32R = mybir.dt.float32r
    BF16 = mybir.dt.bfloat16
    AF = mybir.ActivationFunctionType
    ALU = mybir.AluOpType
    AX = mybir.AxisListType

    NB = N // 128           # 32 token blocks of 128
    NIC = N // 512          # 8 chunks of 512 tokens
    NJT = N // 128          # 32 key tiles of 128

    nc = bass.Bass(trn_type="TRN2", target_bir_lowering=False)

    tok = nc.dram_tensor("tok", [N, D], F32, kind="ExternalInput")
    wq = nc.dram_tensor("wq", [128, 4, 128], F32R, kind="ExternalInput")
    wk = nc.dram_tensor("wk", [128, 4, 128], F32R, kind="ExternalInput")
    wv = nc.dram_tensor("wv", [128, 4, 128], F32R, kind="ExternalInput")
    wo0 = nc.dram_tensor("wo0", [64, 512], F32R, kind="ExternalInput")
    wo1 = nc.dram_tensor("wo1", [64, 512], F32R, kind="ExternalInput")
    out_part = nc.dram_tensor("out_part", [N, D], F32, kind="ExternalOutput")

    tok_r = tok.rearrange("(ic t p) d -> ic p t d", t=4, p=128)
    out_r = out_part.rearrange("(ib t p) e -> ib p t e", t=4, p=128)

    with tile.TileContext(nc) as tc:
        with tc.tile_pool(name="consts", bufs=1) as consts, \
             tc.tile_pool(name="big", bufs=1) as big, \
             tc.tile_pool(name="wpool", bufs=1) as wpool:

            # ---- constants ----
            ident_f = consts.tile([128, 128], F32)
            make_identity(nc, ident_f)
            ident = consts.tile([128, 128], F32R)
            nc.vector.tensor_copy(ident, ident_f)
            eps_t = consts.tile([128, 1], F32)
            nc.vector.memset(eps_t, EPS)
            ones_f = consts.tile([128, 64], F32)
            nc.vector.memset(ones_f, 1.0)
            ones_r = consts.tile([1, 64], F32R)
            nc.vector.tensor_copy(ones_r, ones_f[0:1, :])

            # ---- weights ----
            wq_sb = wpool.tile([128, 4, 128], F32R)
            wk_sb = wpool.tile([128, 4, 128], F32R)
            wv_sb = wpool.tile([128, 4, 128], F32R)
            wo0_sb = wpool.tile([64, 512], F32R)
            wo1_sb = wpool.tile([64, 512], F32R)
            nc.sync.dma_start(out=wq_sb, in_=wq[:, :, :])
            nc.sync.dma_start(out=wk_sb, in_=wk[:, :, :])
            nc.sync.dma_start(out=wv_sb, in_=wv[:, :, :])
            nc.sync.dma_start(out=wo0_sb, in_=wo0[:, :])
            nc.sync.dma_start(out=wo1_sb, in_=wo1[:, :])

            # ---- persistent big buffers ----
            QT = big.tile([128, N], F32R)       # [2 heads x 64 qdims, n]
            KT = big.tile([128, N], F32R)
            Vb = big.tile([128, NJT, 2, 65], F32R)  # per j-tile: [v(64)|ones] per head
            nc.vector.tensor_copy(
                Vb[:, :, :, 64:65],
                ones_f.rearrange("p (a b c) -> p a b c", a=NJT, b=2, c=1))

            GROUPS = []
            jt0 = 0
            while jt0 < NJT:
                g = min(3, NJT - jt0)
                GROUPS.append(list(range(jt0, jt0 + g)))
                jt0 += g
            from contextlib import ExitStack
            outer_ctx = ExitStack()
            ptp = outer_ctx.enter_context(tc.tile_pool(name="pt_pool", bufs=3))
            ops = outer_ctx.enter_context(tc.tile_pool(name="o_psum", bufs=1, space="PSUM"))
            mixps = outer_ctx.enter_context(tc.tile_pool(name="mix_psum", bufs=1, space="PSUM"))
            first_pv = [True]

            def emit_flash_group(o_ps, hl, ib, grp, st):
                h0 = hl * 64
                g = len(grp)
                for k, jt in enumerate(grp):
                    nc.tensor.matmul(
                        st[:, k, :],
                        KT[h0:h0 + 64, jt * 128:(jt + 1) * 128],
                        QT[h0:h0 + 64, ib * 512:(ib + 1) * 512],
                        start=True, stop=True)
                pt = ptp.tile([128, 3, 512], F32R, tag="pt", name="ptg")
                nc.scalar.activation(
                    pt[:, 0:g, :].rearrange("p a b -> p (a b)"),
                    st[:, 0:g, :].rearrange("p a b -> p (a b)"),
                    AF.Exp)
                for k, jt in enumerate(grp):
                    nc.tensor.matmul(
                        o_ps, Vb[:, jt, hl, :], pt[:, k, :],
                        start=first_pv[0], stop=(jt == NJT - 1))
                    first_pv[0] = False

            with tc.tile_pool(name="ab_sbuf", bufs=3) as abp, \
                 tc.tile_pool(name="ab_stats", bufs=4) as stp, \
                 tc.tile_pool(name="ab_psum", bufs=3, space="PSUM") as abps, \
                 tc.tile_pool(name="qk_psum", bufs=2, space="PSUM") as qkps, \
                 tc.tile_pool(name="scr_psum", bufs=1, space="PSUM") as scrps:

                # PE joins: absorb each weight-DMA semaphore with a tiny bf16 matmul
                scr = scrps.tile([2, 2], F32, tag="scr", name="scrj")
                for i, wtile in enumerate((wq_sb, wk_sb, wv_sb, wo0_sb, wo1_sb)):
                    # high bf16 halves of 2 consecutive f32 weights (low halves can be NaN bits)
                    if len(wtile.shape) == 3:
                        src = wtile[0:1, 0:1, 0:2].bitcast(BF16)[:, 0, 1::2]
                    else:
                        src = wtile[0:1, 0:2].bitcast(BF16)[:, 1::2]
                    nc.tensor.matmul(scr, src, src, start=(i == 0), stop=(i == 4))

                for ic in range(NIC):
                    tok4 = abp.tile([128, 4, 512], F32, tag="tok4")
                    nc.gpsimd.dma_start(out=tok4, in_=tok_r[ic])
                    xt = abp.tile([128, 4, 512], F32R, tag="xt")
                    for t in range(4):
                        stats = stp.tile([128, 6], F32, tag="stats")
                        mv = stp.tile([128, 2], F32, tag="mv")
                        ms = stp.tile([128, 1], F32, tag="ms")
                        s_t = stp.tile([128, 1], F32, tag="s_t")
                        rstd = stp.tile([128, 1], F32, tag="rstd")
                        nc.vector.bn_stats(stats, tok4[:, t, :])
                        nc.vector.bn_aggr(mv, stats)
                        # E[x^2] = mean^2 + var
                        nc.vector.scalar_tensor_tensor(
                            ms, mv[:, 0:1], mv[:, 0:1], mv[:, 1:2],
                            op0=ALU.mult, op1=ALU.add)
                        nc.scalar.activation(s_t, ms, AF.Sqrt, bias=eps_t, scale=1.0)
                        nc.vector.reciprocal(rstd, s_t)
                        # alternate the scale between DVE and GpSimd to
                        # balance the two phase-A bottleneck engines; per-t
                        # tiles so each transpose depends only on its own scale
                        xn = stp.tile([128, 512], F32R, tag="xn")
                        eng = nc.vector if t % 2 == 0 else nc.gpsimd
                        eng.tensor_scalar_mul(xn, in0=tok4[:, t, :], scalar1=rstd)
                        tp = abps.tile([128, 4, 128], F32R, tag="tp")
                        for c in range(4):
                            nc.tensor.transpose(tp[:, c, :], xn[:, c * 128:(c + 1) * 128], ident)
                        nc.scalar.copy(xt[:, :, t * 128:(t + 1) * 128], tp)

                    # QKV^T for this 512-token chunk (V first: see DVE ordering note)
                    vt = abp.tile([128, 512], F32R, tag="vt")
                    for wtile, dst in ((wv_sb, None), (wq_sb, QT), (wk_sb, KT)):
                        ps = qkps.tile([128, 512], F32, tag="qk")
                        for c in range(4):
                            nc.tensor.matmul(ps, wtile[:, c, :], xt[:, c, :],
                                             start=(c == 0), stop=(c == 3))
                        if dst is None:
                            nc.vector.tensor_copy(vt, ps)
                        else:
                            nc.vector.tensor_copy(dst[:, ic * 512:(ic + 1) * 512], ps)
                    # V^T -> V natural into Vb (j on partitions), all 4 j-tiles
                    vtp = abps.tile([128, 4, 128], F32R, tag="tp")
                    for jl in range(4):
                        nc.tensor.transpose(vtp[:, jl, :], vt[:, jl * 128:(jl + 1) * 128], ident)
                    nc.vector.tensor_copy(
                        Vb[:, ic * 4:(ic + 1) * 4, :, 0:64],
                        vtp.rearrange("p jl (h v) -> p jl h v", h=2))

            # ---- phase C + D ----
            with tc.tile_pool(name="c_sbuf", bufs=2) as cp, \
                 tc.tile_pool(name="osb_pool", bufs=2) as osbp, \
                 tc.tile_pool(name="lc_pool", bufs=2) as lcp, \
                 tc.tile_pool(name="st_psum", bufs=2, space="PSUM") as stps:

                saved = {}

                def emit_tail(o_ps, ib, hl):
                    # l = row 64 of o_ps; o^T = rows 0..63. 1/l is folded into
                    # the out-projection extraction (per-partition scalar),
                    # so we need l transposed to [i-partitions, 1] — done with
                    # a small SBUF->SBUF DMA scatter.
                    l_sb = cp.tile([1, 512], F32, tag="l_sb")
                    nc.vector.tensor_copy(l_sb, o_ps[64:65, :])
                    o_sb = osbp.tile([64, 512], F32R, tag=f"osb{hl}")
                    nc.vector.tensor_copy(o_sb, o_ps[0:64, :])
                    linv_pre = cp.tile([128, 4], F32, tag="linv_pre")
                    for it in range(4):
                        nc.sync.dma_start(
                            out=linv_pre[:, it:it + 1],
                            in_=l_sb[0:1, it * 128:(it + 1) * 128])
                    linv_col = lcp.tile([128, 4], F32, tag=f"lc{hl}")
                    nc.vector.reciprocal(linv_col, linv_pre)
                    saved[(ib, hl)] = (o_sb, linv_col)

                def emit_outproj_tile(ib, it, out_sb, tmp_on_act=False,
                                      use_st=False):
                    o_sb0, lc0 = saved[(ib, 0)]
                    o_sb1, lc1 = saved[(ib, 1)]
                    pool = stps if use_st else mixps
                    tg = "st" if use_st else "mix"
                    op_ps = pool.tile([128, 512], F32, tag=tg)
                    nc.tensor.matmul(op_ps, o_sb0[:, it * 128:(it + 1) * 128],
                                     wo0_sb, start=True, stop=True)
                    tmp = cp.tile([128, 512], F32, tag="tmp")
                    if tmp_on_act:
                        nc.scalar.mul(tmp, op_ps, lc0[:, it:it + 1])
                    else:
                        nc.vector.tensor_scalar_mul(tmp, in0=op_ps, scalar1=lc0[:, it:it + 1])
                    op_ps2 = pool.tile([128, 512], F32, tag=tg)
                    nc.tensor.matmul(op_ps2, o_sb1[:, it * 128:(it + 1) * 128],
                                     wo1_sb, start=True, stop=True)
                    nc.vector.scalar_tensor_tensor(
                        out_sb[:, it, :], op_ps2, lc1[:, it:it + 1], tmp,
                        op0=ALU.mult, op1=ALU.add)

                def emit_st(ib, hl, grp):
                    h0 = hl * 64
                    st = stps.tile([128, 3, 512], F32, tag="st", name="stg")
                    for k, jt in enumerate(grp):
                        nc.tensor.matmul(
                            st[:, k, :],
                            KT[h0:h0 + 64, jt * 128:(jt + 1) * 128],
                            QT[h0:h0 + 64, ib * 512:(ib + 1) * 512],
                            start=True, stop=True)
                    return st

                prev = None
                pending_op = None  # i-block whose out-projection is owed
                out_sbs = {}
                iters = [(ib, hl) for ib in range(NIC) for hl in range(2)]
                # S^T groups are emitted one step ahead of exp/PV so the PE
                # keeps ACT fed across group and iteration boundaries
                st_cur = emit_st(iters[0][0], iters[0][1], GROUPS[0])
                for idx, (ib, hl) in enumerate(iters):
                    o_ps = ops.tile([65, 512], F32, tag="o")
                    first_pv[0] = True
                    for g_idx, grp in enumerate(GROUPS):
                        g = len(grp)
                        st = st_cur
                        pt = ptp.tile([128, 3, 512], F32R, tag="pt", name="ptg")
                        nc.scalar.activation(
                            pt[:, 0:g, :].rearrange("p a b -> p (a b)"),
                            st[:, 0:g, :].rearrange("p a b -> p (a b)"),
                            AF.Exp)
                        if g_idx + 1 < len(GROUPS):
                            st_cur = emit_st(ib, hl, GROUPS[g_idx + 1])
                        elif idx + 1 < len(iters):
                            st_cur = emit_st(iters[idx + 1][0], iters[idx + 1][1],
                                             GROUPS[0])
                        # software-pipelined: previous iteration's tail
                        # BEFORE this iteration's first PV (which reuses the
                        # single o bank and must wait for the tail's reads)
                        if g_idx == 0 and prev is not None:
                            emit_tail(*prev)
                            if prev[2] == 1:
                                pending_op = prev[1]
                                out_sbs[pending_op] = cp.tile(
                                    [128, 4, 512], F32, tag="out_sb",
                                    name=f"outsb{pending_op}")
                        elif 1 <= g_idx <= 4 and pending_op is not None:
                            emit_outproj_tile(pending_op, g_idx - 1,
                                              out_sbs[pending_op])
                            if g_idx == 4:
                                done = pending_op
                                nc.sync.dma_start(out=out_r[done],
                                                  in_=out_sbs.pop(done))
                                saved.pop((done, 0))
                                saved.pop((done, 1))
                                pending_op = None
                        for k, jt in enumerate(grp):
                            nc.tensor.matmul(
                                o_ps, Vb[:, jt, hl, :], pt[:, k, :],
                                start=first_pv[0], stop=(jt == NJT - 1))
                            first_pv[0] = False
                    prev = (o_ps, ib, hl)

                # final tail + out-projection (use the now-free st slots so the
                # 8 matmuls don't serialize on one PSUM bank)
                emit_tail(*prev)
                fin = prev[1]
                out_fin = cp.tile([128, 4, 512], F32, tag="out_sb")
                for it in range(4):
                    emit_outproj_tile(fin, it, out_fin, tmp_on_act=True,
                                      use_st=True)
                nc.sync.dma_start(out=out_r[fin], in_=out_fin)
            outer_ctx.close()

    fix_waits_nc(nc, mybir)
    return nc


def fix_waits_nc(nc, mybir):
    """Post-pass over the scheduled program: (1) remove semaphore waits that
    are transitively implied by earlier waits (Tile emits per-proc-minimal,
    not transitively-minimal, waits), (2) split any instruction still
    carrying more than one wait by injecting single-wait NoOps in front of
    it — this walrus build rejects >1 sync wait per instruction.
    Mutates nc in place so CoreSim and hardware run identical sync."""
    nop_id = [0]

    def _is_ge(w):
        return w.sync_type == "semaphore" and w.wait_mode == "sem-ge-imm"

    for fn in nc.m.functions:
        for blk in fn.blocks:
            insts = list(blk.instructions)
            n = len(insts)

            producers = {}
            cum = {}
            nonmono = set()  # sems ever decremented: counter logic invalid
            for idx, inst in enumerate(insts):
                si = inst.sync_info
                for u in (si.on_update if si else []) or []:
                    if u.sync_type != "semaphore":
                        continue
                    sid = u.id
                    if u.update_mode != "sem-inc":
                        nonmono.add(sid)
                        continue
                    cum[sid] = cum.get(sid, 0) + int(u.update_value)
                    producers.setdefault(sid, []).append((cum[sid], idx))

            def producer_of(sid, val):
                for cv, idx in producers.get(sid, ()):
                    if cv >= val:
                        return idx
                return None

            prev_eng = [None] * n
            last = {}
            for idx, inst in enumerate(insts):
                e = inst.engine
                prev_eng[idx] = last.get(e)
                last[e] = idx

            def get_waits(inst):
                si = inst.sync_info
                return list(si.on_wait) if si and si.on_wait else []

            def is_ge(w):
                return _is_ge(w) and w.id not in nonmono

            know = [dict() for _ in range(n)]
            for _ in range(3):
                changed = False
                for idx, inst in enumerate(insts):
                    k = dict(know[prev_eng[idx]]) if prev_eng[idx] is not None else {}
                    for w in get_waits(inst):
                        if not is_ge(w):
                            continue
                        sid, val = w.id, int(w.wait_value)
                        if k.get(sid, -1) < val:
                            k[sid] = val
                        p = producer_of(sid, val)
                        if p is not None:
                            for s2, v2 in know[p].items():
                                if k.get(s2, -1) < v2:
                                    k[s2] = v2
                    if k != know[idx]:
                        know[idx] = k
                        changed = True
                if not changed:
                    break

            new_insts = []
            dirty = False
            for idx, inst in enumerate(insts):
                si = inst.sync_info
                waits = get_waits(inst)
                if si is not None and waits:
                    base = dict(know[prev_eng[idx]]) if prev_eng[idx] is not None else {}
                    kept = []
                    for w in waits:
                        if is_ge(w):
                            sid, val = w.id, int(w.wait_value)
                            if base.get(sid, -1) >= val:
                                continue
                            base[sid] = val
                            p = producer_of(sid, val)
                            if p is not None:
                                for s2, v2 in know[p].items():
                                    if base.get(s2, -1) < v2:
                                        base[s2] = v2
                        kept.append(w)
                    if len(kept) != len(waits) or len(kept) > 1:
                        dirty = True
                        for w in kept[:-1]:
                            nop_id[0] += 1
                            nop = mybir.InstNoOp(
                                name=f"I-waitfix-{nop_id[0]}", ins=[], outs=[])
                            nop.engine = inst.engine
                            nop.sync_info = mybir.SyncInfo(on_wait=[w], on_update=[])
                            nc.register_instruction(nop)
                            new_insts.append(nop)
                        inst.sync_info = mybir.SyncInfo(
                            on_wait=kept[-1:],
                            on_update=list(si.on_update or []))
                new_insts.append(inst)
            if dirty:
                blk.instructions = new_insts


def get_program():
    if "nc" not in _prog_cache:
        _prog_cache["nc"] = build_program()
    return _prog_cache["nc"]


def _prep_inputs(tokens, norm_weight, w_qkv, w_out):
    tokens = np.ascontiguousarray(np.asarray(tokens, dtype=np.float32))
    norm_weight = np.asarray(norm_weight, dtype=np.float32)
    w_qkv = np.asarray(w_qkv, dtype=np.float32)
    w_out = np.asarray(w_out, dtype=np.float32)

    wp = w_qkv * norm_weight[:, None]  # fold RMSNorm weight into qkv weights

    in_maps = []
    for c in range(NCORES):
        b = c // 4
        h0 = 2 * (c % 4)
        cols = np.r_[h0 * DH:(h0 + 1) * DH, (h0 + 1) * DH:(h0 + 2) * DH]
        m = {}
        m["tok"] = tokens[b]
        for name, off in (("wq", 0), ("wk", DI), ("wv", 2 * DI)):
            w = wp[:, off + h0 * DH: off + (h0 + 2) * DH]       # [512, 128]
            m[name] = np.ascontiguousarray(
                w.reshape(4, 128, 128).transpose(1, 0, 2))       # [128, 4, 128]
        rows = w_out[h0 * DH:(h0 + 2) * DH, :]                   # [128, 512]
        m["wo0"] = np.ascontiguousarray(rows[0:64])
        m["wo1"] = np.ascontiguousarray(rows[64:128])
        in_maps.append(m)
    return in_maps


def run(tokens, norm_weight, w_qkv, w_out, trace=False):
    from concourse.bass_utils import run_bass_kernel_spmd
    nc = get_program()
    in_maps = _prep_inputs(tokens, norm_weight, w_qkv, w_out)
    res = run_bass_kernel_spmd(nc, in_maps, core_ids=list(range(NCORES)), trace=trace)
    parts = [res.results[c]["out_part"] for c in range(NCORES)]
    out = np.empty((B, N, D), dtype=np.float32)
    for b in range(B):
        out[b] = parts[4 * b] + parts[4 * b + 1] + parts[4 * b + 2] + parts[4 * b + 3]
    return out, res


def kernel(tokens, norm_weight, w_qkv, w_out):
    out, _ = run(tokens, norm_weight, w_qkv, w_out, trace=False)
    return out
